# revision 9
# baseline (speedup 1.0000x reference)
"""Trainium2 Bass kernel for nn_CAKernel_47459388621075.

10 steps of x = clip(x + 0.1*relu(conv5x5_circular(x, W)), 0, 1) on
x:(16,3,1024,1024) f32, W:(3,3,5,5) f32.

Sharding: batch-parallel over 8 NeuronCores (2 images/core); the circular
conv is per-image so no cross-core traffic.

Per-core strategy: the whole state lives in SBUF in fp16 for all 10 steps
(per core 2*3*1024*1024*2B = 12.6 MB), eliminating the per-step HBM
round-trip that made the v1 kernel DMA-bound and kept the PE
HAM-throttled at 1.2 GHz. H is split into 27 row blocks (26x38 + 36);
each block owns a persistent window tile [3*(B+4), 1028] in row-major
interleaved partition order p = 3*row + channel, holding B interior rows
plus 2+2 halo rows (so each halo is 6 contiguous partitions -> one
SBUF->SBUF DMA per exchange direction) and 2+2 circular column halos.
The 5x5x3x3 conv runs as 5 PSUM-accumulated fp16 matmuls (one per
kernel column dx) per 512-col group; the banded stationary
[3(B+4), 3(B+4)] maps out partition m = 3*(r+2)+co so PSUM/ACT output
is partition-aligned with the state tile (halo partitions get zeros, so
the elementwise tail can run on all partitions uniformly). Per block:
ACT t = relu(0.1*psum) -> DVE t2 = t + x -> DVE x' = min(t2, 1) written
back into the state tile, DVE refreshes the 4 column-halo columns, and
two Sync-queue DMAs push the boundary rows into the neighbors' halo
partitions. Block order rotates by +3 each step so next-step blocks
never wait on the tail of the previous step and the PE stays warm.
"""
import sys

sys.path.insert(0, "/opt/trn_rl_repo")

import numpy as np

N_CORES = 8
CG = 512  # matmul column group (one PSUM bank of f32)


def block_sizes(H):
    """Split H into blocks of 38 (3*(38+4)=126 <= 128 partitions)."""
    bs = []
    rem = H
    while rem > 42:
        bs.append(38)
        rem -= 38
    bs.append(rem)
    assert sum(bs) == H and all(6 <= b <= 42 for b in bs)
    return bs


def make_lhsT(W: np.ndarray, B: int) -> np.ndarray:
    """Stationary [3*(B+4), 5*3*(B+4)] fp16, one 3(B+4)-col band per dx.

    Window partition p = 3*(r+dy) + ci; out column m = 3*(r+2) + co (the
    first/last 6 columns stay zero so PSUM rows align with the state
    tile's partition layout and halo partitions receive zeros).
    """
    assert W.shape == (3, 3, 5, 5)
    S = B + 4
    KP = 3 * S
    lhsT = np.zeros((KP, 5, KP), dtype=np.float32)
    for ci in range(3):
        for r in range(B):
            for dy in range(5):
                for dx in range(5):
                    for co in range(3):
                        lhsT[3 * (r + dy) + ci, dx, 3 * (r + 2) + co] = W[
                            co, ci, dy, dx
                        ]
    return lhsT.reshape(KP, 5 * KP).astype(np.float16)


def build_body(tc, x_ap, lh_aps, y_ap, n_img, H, Wc, steps):
    """Emit the Tile program. x_ap:(n_img,3,H,Wc) f32 in; y_ap same out."""
    from contextlib import ExitStack

    from concourse import mybir

    nc = tc.nc
    f16 = mybir.dt.float16
    f32 = mybir.dt.float32
    Relu = mybir.ActivationFunctionType.Relu

    BS = block_sizes(H)
    nb = len(BS)
    OFF = np.concatenate([[0], np.cumsum(BS)])
    assert Wc % CG == 0 or Wc < CG
    n_cg = max(1, Wc // CG)
    cg = Wc // n_cg
    WF = Wc + 4

    # x/y with partition-major (row, channel) order
    xr = [x_ap[img].rearrange("c h w -> h c w") for img in range(n_img)]
    yr = [y_ap[img].rearrange("c h w -> h c w") for img in range(n_img)]

    ctx = ExitStack()
    const_pool = ctx.enter_context(tc.tile_pool(name="const", bufs=len(lh_aps)))
    state_pool = ctx.enter_context(tc.tile_pool(name="state", bufs=1))
    stage_pool = ctx.enter_context(tc.tile_pool(name="stage", bufs=6))
    act_pool = ctx.enter_context(tc.tile_pool(name="act", bufs=8))
    psum_pool = ctx.enter_context(tc.tile_pool(name="psum", bufs=8, space="PSUM"))

    lh = {}
    for Bsz, ap in lh_aps.items():
        KP = 3 * (Bsz + 4)
        t = const_pool.tile([KP, 5 * KP], f16)
        nc.sync.dma_start(t[:], ap[:, :])
        lh[Bsz] = t

    st = {}
    for img in range(n_img):
        for b in range(nb):
            KP = 3 * (BS[b] + 4)
            st[img, b] = state_pool.tile(
                [KP, WF], f16, name=f"st{img}_{b}", tag=f"st{img}_{b}"
            )

    def load_block(img, b):
        B = BS[b]
        KP = 3 * (B + 4)
        O0 = int(OFF[b])
        stg = stage_pool.tile([126, Wc], f32, tag="stage")
        if b == 0:
            nc.gpsimd.dma_start(stg[6:KP, :], xr[img][0 : B + 2, :, :])
            nc.gpsimd.dma_start(stg[0:6, :], xr[img][H - 2 : H, :, :])
        elif b == nb - 1:
            nc.gpsimd.dma_start(stg[0 : KP - 6, :], xr[img][O0 - 2 : H, :, :])
            nc.gpsimd.dma_start(stg[KP - 6 : KP, :], xr[img][0:2, :, :])
        else:
            nc.gpsimd.dma_start(stg[0:KP, :], xr[img][O0 - 2 : O0 + B + 2, :, :])
        tile = st[img, b]
        nc.vector.tensor_copy(tile[0:KP, 2 : Wc + 2], stg[0:KP, :])
        nc.vector.tensor_copy(tile[0:KP, 0:2], tile[0:KP, Wc : Wc + 2])
        nc.vector.tensor_copy(tile[0:KP, Wc + 2 : Wc + 4], tile[0:KP, 2:4])

    def emit_block(s, img, b, last):
        B = BS[b]
        KP = 3 * (B + 4)
        O0 = int(OFF[b])
        tile = st[img, b]
        lht = lh[B]
        ts = []
        for g in range(n_cg):
            psum = psum_pool.tile([126, cg], f32, tag="ps")
            for dx in range(5):
                nc.tensor.matmul(
                    psum[0:KP, :],
                    lht[:, dx * KP : (dx + 1) * KP],
                    tile[0:KP, g * cg + dx : g * cg + dx + cg],
                    start=(dx == 0),
                    stop=(dx == 4),
                )
            t = act_pool.tile([126, cg], f16, tag="t")
            nc.scalar.activation(t[0:KP, :], psum[0:KP, :], Relu, scale=0.1)
            ts.append(t)
        for g in range(n_cg):
            t2 = act_pool.tile([126, cg], f16, tag="t2")
            nc.vector.tensor_add(
                t2[0:KP, :], ts[g][0:KP, :], tile[0:KP, g * cg + 2 : (g + 1) * cg + 2]
            )
            if last:
                stg = stage_pool.tile([126, cg], f32, tag="stout")
                nc.vector.tensor_scalar_min(stg[0:KP, :], t2[0:KP, :], 1.0)
                q = nc.sync if g == 0 else nc.gpsimd
                q.dma_start(
                    yr[img][O0 : O0 + B, :, g * cg : (g + 1) * cg],
                    stg[6 : 6 + 3 * B, :],
                )
            else:
                nc.vector.tensor_scalar_min(
                    tile[0:KP, g * cg + 2 : (g + 1) * cg + 2], t2[0:KP, :], 1.0
                )
                if g == 0:
                    nc.vector.tensor_copy(
                        tile[0:KP, Wc + 2 : Wc + 4], tile[0:KP, 2:4]
                    )
                if g == n_cg - 1:
                    nc.vector.tensor_copy(tile[0:KP, 0:2], tile[0:KP, Wc : Wc + 2])

    def emit_pushes(img, a):
        """Exchange boundary rows between blocks a and a+1 (mod nb)."""
        b = (a + 1) % nb
        Ba = BS[a]
        ta, tb = st[img, a], st[img, b]
        # b's upper halo (rows O_b-2, O_b-1) <- a's last two rows
        nc.sync.dma_start(tb[0:6, :], ta[3 * Ba : 3 * Ba + 6, :])
        # a's lower halo (rows O_a+Ba, +Ba+1) <- b's first two rows
        nc.sync.dma_start(ta[3 * Ba + 6 : 3 * Ba + 12, :], tb[6:12, :])

    for i in range(nb):
        for img in range(n_img):
            load_block(img, i)

    for s in range(steps):
        last = s == steps - 1
        rot = (3 * s) % nb
        order = [(rot + i) % nb for i in range(nb)]
        for i, b in enumerate(order):
            for img in range(n_img):
                emit_block(s, img, b, last)
            if not last and i >= 1:
                for img in range(n_img):
                    emit_pushes(img, (b - 1) % nb)
        if not last:
            for img in range(n_img):
                emit_pushes(img, (order[0] - 1) % nb)

    ctx.close()


_PROGRAM_CACHE = {}


def _build_program(n_img, H, Wc, steps):
    key = (n_img, H, Wc, steps)
    if key in _PROGRAM_CACHE:
        return _PROGRAM_CACHE[key]
    import concourse.tile as tile
    from concourse import bacc, mybir

    nc = bacc.Bacc(
        "TRN2",
        target_bir_lowering=False,
        debug=False,
        enable_asserts=False,
        num_devices=N_CORES,
    )
    f16 = mybir.dt.float16
    f32 = mybir.dt.float32
    x_ap = nc.dram_tensor("x", (n_img, 3, H, Wc), f32, kind="ExternalInput").ap()
    lh_aps = {}
    for Bsz in sorted(set(block_sizes(H))):
        KP = 3 * (Bsz + 4)
        lh_aps[Bsz] = nc.dram_tensor(
            f"lhsT{Bsz}", (KP, 5 * KP), f16, kind="ExternalInput"
        ).ap()
    y_ap = nc.dram_tensor("y", (n_img, 3, H, Wc), f32, kind="ExternalOutput").ap()
    with tile.TileContext(nc) as tc:
        build_body(tc, x_ap, lh_aps, y_ap, n_img, H, Wc, steps)
    nc.compile()
    _PROGRAM_CACHE[key] = nc
    return nc


def _make_in_maps(x, W, n_cores=N_CORES):
    n, c, H, Wc = x.shape
    per = n // n_cores
    lhs = {Bsz: make_lhsT(W, Bsz) for Bsz in sorted(set(block_sizes(H)))}
    return [
        dict(
            {"x": x[i * per : (i + 1) * per]},
            **{f"lhsT{B}": l for B, l in lhs.items()},
        )
        for i in range(n_cores)
    ]


def kernel(x: np.ndarray, W: np.ndarray, steps) -> np.ndarray:
    from concourse.bass_utils import run_bass_kernel_spmd

    x = np.ascontiguousarray(np.asarray(x), dtype=np.float32)
    W = np.asarray(W, dtype=np.float32)
    steps = int(steps)
    n, c, H, Wc = x.shape
    assert c == 3 and n % N_CORES == 0
    per = n // N_CORES

    nc = _build_program(per, H, Wc, steps)
    in_maps = _make_in_maps(x, W)
    res = run_bass_kernel_spmd(nc, in_maps, core_ids=list(range(N_CORES)))
    out = np.concatenate([res.results[i]["y"] for i in range(N_CORES)], axis=0)
    return out.astype(np.float32)


# revision 10
# speedup vs baseline: 1.1514x; 1.1514x over previous
"""Trainium2 Bass kernel for nn_CAKernel_47459388621075.

10 steps of x = clip(x + 0.1*relu(conv5x5_circular(x, W)), 0, 1) on
x:(16,3,1024,1024) f32, W:(3,3,5,5) f32.

Sharding: batch-parallel over 8 NeuronCores (2 images/core); the circular
conv is per-image so no cross-core traffic.

Per-core strategy: the whole state lives in SBUF in fp16 for all 10 steps
(per core 2*3*1024*1024*2B = 12.6 MB), eliminating the per-step HBM
round-trip that made the v1 kernel DMA-bound and kept the PE
HAM-throttled at 1.2 GHz. H is split into 27 row blocks (26x38 + 36);
each block owns a persistent window tile [3*(B+4), 1028] in row-major
interleaved partition order p = 3*row + channel, holding B interior rows
plus 2+2 halo rows (so each halo is 6 contiguous partitions -> one
SBUF->SBUF DMA per exchange direction) and 2+2 circular column halos.
The 5x5x3x3 conv runs as 5 PSUM-accumulated fp16 matmuls (one per
kernel column dx) per 512-col group; the banded stationary
[3(B+4), 3(B+4)] maps out partition m = 3*(r+2)+co so PSUM/ACT output
is partition-aligned with the state tile (halo partitions get zeros, so
the elementwise tail can run on all partitions uniformly). Per block:
ACT t = relu(0.1*psum) -> DVE t2 = t + x -> DVE x' = min(t2, 1) written
back into the state tile, DVE refreshes the 4 column-halo columns, and
two Sync-queue DMAs push the boundary rows into the neighbors' halo
partitions. Block order rotates by +3 each step so next-step blocks
never wait on the tail of the previous step and the PE stays warm.
"""
import sys

sys.path.insert(0, "/opt/trn_rl_repo")

import numpy as np

N_CORES = 8
CG = 512  # matmul column group (one PSUM bank of f32)


def block_sizes(H):
    """Split H into blocks of 38 (3*(38+4)=126 <= 128 partitions)."""
    bs = []
    rem = H
    while rem > 42:
        bs.append(38)
        rem -= 38
    bs.append(rem)
    assert sum(bs) == H and all(6 <= b <= 42 for b in bs)
    return bs


def make_lhsT(W: np.ndarray, B: int) -> np.ndarray:
    """Stationary [3*(B+4), 5*3*(B+4)] fp16, one 3(B+4)-col band per dx.

    Window partition p = 3*(r+dy) + ci; out column m = 3*(r+2) + co (the
    first/last 6 columns stay zero so PSUM rows align with the state
    tile's partition layout and halo partitions receive zeros).
    """
    assert W.shape == (3, 3, 5, 5)
    S = B + 4
    KP = 3 * S
    lhsT = np.zeros((KP, 5, KP), dtype=np.float32)
    for ci in range(3):
        for r in range(B):
            for dy in range(5):
                for dx in range(5):
                    for co in range(3):
                        lhsT[3 * (r + dy) + ci, dx, 3 * (r + 2) + co] = W[
                            co, ci, dy, dx
                        ]
    return lhsT.reshape(KP, 5 * KP).astype(np.float16)


def build_body(tc, x_ap, lh_aps, y_ap, n_img, H, Wc, steps):
    """Emit the Tile program. x_ap:(n_img,3,H,Wc) f32 in; y_ap same out."""
    from contextlib import ExitStack

    from concourse import mybir

    nc = tc.nc
    f16 = mybir.dt.float16
    f32 = mybir.dt.float32
    Relu = mybir.ActivationFunctionType.Relu

    BS = block_sizes(H)
    nb = len(BS)
    OFF = np.concatenate([[0], np.cumsum(BS)])
    assert Wc % CG == 0 or Wc < CG
    n_cg = max(1, Wc // CG)
    cg = Wc // n_cg
    WF = Wc + 4

    # x/y with partition-major (row, channel) order
    xr = [x_ap[img].rearrange("c h w -> h c w") for img in range(n_img)]
    yr = [y_ap[img].rearrange("c h w -> h c w") for img in range(n_img)]

    ctx = ExitStack()
    const_pool = ctx.enter_context(tc.tile_pool(name="const", bufs=len(lh_aps)))
    state_pool = ctx.enter_context(tc.tile_pool(name="state", bufs=1))
    stage_pool = ctx.enter_context(tc.tile_pool(name="stage", bufs=6))
    act_pool = ctx.enter_context(tc.tile_pool(name="act", bufs=8))
    psum_pool = ctx.enter_context(tc.tile_pool(name="psum", bufs=8, space="PSUM"))

    lh = {}
    for Bsz, ap in lh_aps.items():
        KP = 3 * (Bsz + 4)
        t = const_pool.tile([KP, 5 * KP], f16)
        nc.sync.dma_start(t[:], ap[:, :])
        lh[Bsz] = t

    st = {}
    for img in range(n_img):
        for b in range(nb):
            KP = 3 * (BS[b] + 4)
            st[img, b] = state_pool.tile(
                [KP, WF], f16, name=f"st{img}_{b}", tag=f"st{img}_{b}"
            )

    def load_block(img, b):
        B = BS[b]
        KP = 3 * (B + 4)
        O0 = int(OFF[b])
        stg = stage_pool.tile([126, Wc], f32, tag="stage")
        if b == 0:
            nc.gpsimd.dma_start(stg[6:KP, :], xr[img][0 : B + 2, :, :])
            nc.gpsimd.dma_start(stg[0:6, :], xr[img][H - 2 : H, :, :])
        elif b == nb - 1:
            nc.gpsimd.dma_start(stg[0 : KP - 6, :], xr[img][O0 - 2 : H, :, :])
            nc.gpsimd.dma_start(stg[KP - 6 : KP, :], xr[img][0:2, :, :])
        else:
            nc.gpsimd.dma_start(stg[0:KP, :], xr[img][O0 - 2 : O0 + B + 2, :, :])
        tile = st[img, b]
        nc.vector.tensor_copy(tile[0:KP, 2 : Wc + 2], stg[0:KP, :])
        nc.vector.tensor_copy(tile[0:KP, 0:2], tile[0:KP, Wc : Wc + 2])
        nc.vector.tensor_copy(tile[0:KP, Wc + 2 : Wc + 4], tile[0:KP, 2:4])

    def emit_block(s, img, b, last):
        B = BS[b]
        KP = 3 * (B + 4)
        O0 = int(OFF[b])
        tile = st[img, b]
        lht = lh[B]
        t = act_pool.tile([126, Wc], f16, tag="t")
        for g in range(n_cg):
            psum = psum_pool.tile([126, cg], f32, tag="ps")
            for dx in range(5):
                nc.tensor.matmul(
                    psum[0:KP, :],
                    lht[:, dx * KP : (dx + 1) * KP],
                    tile[0:KP, g * cg + dx : g * cg + dx + cg],
                    start=(dx == 0),
                    stop=(dx == 4),
                )
            nc.scalar.activation(
                t[0:KP, g * cg : (g + 1) * cg], psum[0:KP, :], Relu, scale=0.1
            )
        t2 = act_pool.tile([126, Wc], f16, tag="t2")
        nc.vector.tensor_add(t2[0:KP, :], t[0:KP, :], tile[0:KP, 2 : Wc + 2])
        if last:
            stg = stage_pool.tile([126, Wc], f32, tag="stage")
            nc.vector.tensor_scalar_min(stg[0:KP, :], t2[0:KP, :], 1.0)
            for g in range(n_cg):
                q = nc.sync if g % 2 == 0 else nc.gpsimd
                q.dma_start(
                    yr[img][O0 : O0 + B, :, g * cg : (g + 1) * cg],
                    stg[6 : 6 + 3 * B, g * cg : (g + 1) * cg],
                )
        else:
            nc.vector.tensor_scalar_min(tile[0:KP, 2 : Wc + 2], t2[0:KP, :], 1.0)
            nc.vector.tensor_copy(tile[0:KP, 0:2], tile[0:KP, Wc : Wc + 2])
            nc.vector.tensor_copy(tile[0:KP, Wc + 2 : Wc + 4], tile[0:KP, 2:4])

    def emit_pushes(img, a):
        """Exchange boundary rows between blocks a and a+1 (mod nb)."""
        b = (a + 1) % nb
        Ba = BS[a]
        ta, tb = st[img, a], st[img, b]
        # b's upper halo (rows O_b-2, O_b-1) <- a's last two rows
        nc.sync.dma_start(tb[0:6, :], ta[3 * Ba : 3 * Ba + 6, :])
        # a's lower halo (rows O_a+Ba, +Ba+1) <- b's first two rows
        nc.sync.dma_start(ta[3 * Ba + 6 : 3 * Ba + 12, :], tb[6:12, :])

    for i in range(nb):
        for img in range(n_img):
            load_block(img, i)

    for s in range(steps):
        last = s == steps - 1
        rot = (3 * s) % nb
        order = [(rot + i) % nb for i in range(nb)]
        for i, b in enumerate(order):
            for img in range(n_img):
                emit_block(s, img, b, last)
            if not last and i >= 1:
                for img in range(n_img):
                    emit_pushes(img, (b - 1) % nb)
        if not last:
            for img in range(n_img):
                emit_pushes(img, (order[0] - 1) % nb)

    ctx.close()


_PROGRAM_CACHE = {}


def _build_program(n_img, H, Wc, steps):
    key = (n_img, H, Wc, steps)
    if key in _PROGRAM_CACHE:
        return _PROGRAM_CACHE[key]
    import concourse.tile as tile
    from concourse import bacc, mybir

    nc = bacc.Bacc(
        "TRN2",
        target_bir_lowering=False,
        debug=False,
        enable_asserts=False,
        num_devices=N_CORES,
    )
    f16 = mybir.dt.float16
    f32 = mybir.dt.float32
    x_ap = nc.dram_tensor("x", (n_img, 3, H, Wc), f32, kind="ExternalInput").ap()
    lh_aps = {}
    for Bsz in sorted(set(block_sizes(H))):
        KP = 3 * (Bsz + 4)
        lh_aps[Bsz] = nc.dram_tensor(
            f"lhsT{Bsz}", (KP, 5 * KP), f16, kind="ExternalInput"
        ).ap()
    y_ap = nc.dram_tensor("y", (n_img, 3, H, Wc), f32, kind="ExternalOutput").ap()
    with tile.TileContext(nc) as tc:
        build_body(tc, x_ap, lh_aps, y_ap, n_img, H, Wc, steps)
    nc.compile()
    _PROGRAM_CACHE[key] = nc
    return nc


def _make_in_maps(x, W, n_cores=N_CORES):
    n, c, H, Wc = x.shape
    per = n // n_cores
    lhs = {Bsz: make_lhsT(W, Bsz) for Bsz in sorted(set(block_sizes(H)))}
    return [
        dict(
            {"x": x[i * per : (i + 1) * per]},
            **{f"lhsT{B}": l for B, l in lhs.items()},
        )
        for i in range(n_cores)
    ]


def kernel(x: np.ndarray, W: np.ndarray, steps) -> np.ndarray:
    from concourse.bass_utils import run_bass_kernel_spmd

    x = np.ascontiguousarray(np.asarray(x), dtype=np.float32)
    W = np.asarray(W, dtype=np.float32)
    steps = int(steps)
    n, c, H, Wc = x.shape
    assert c == 3 and n % N_CORES == 0
    per = n // N_CORES

    nc = _build_program(per, H, Wc, steps)
    in_maps = _make_in_maps(x, W)
    res = run_bass_kernel_spmd(nc, in_maps, core_ids=list(range(N_CORES)))
    out = np.concatenate([res.results[i]["y"] for i in range(N_CORES)], axis=0)
    return out.astype(np.float32)


# revision 11
# speedup vs baseline: 1.1766x; 1.0219x over previous
"""Trainium2 Bass kernel for nn_CAKernel_47459388621075.

10 steps of x = clip(x + 0.1*relu(conv5x5_circular(x, W)), 0, 1) on
x:(16,3,1024,1024) f32, W:(3,3,5,5) f32.

Sharding: batch-parallel over 8 NeuronCores (2 images/core); the circular
conv is per-image so no cross-core traffic.

Per-core strategy: the whole state lives in SBUF in fp16 for all 10 steps
(per core 2*3*1024*1024*2B = 12.6 MB), eliminating the per-step HBM
round-trip that made the v1 kernel DMA-bound and kept the PE
HAM-throttled at 1.2 GHz. H is split into 27 row blocks (26x38 + 36);
each block owns a persistent window tile [3*(B+4), 1028] in row-major
interleaved partition order p = 3*row + channel, holding B interior rows
plus 2+2 halo rows (so each halo is 6 contiguous partitions -> one
SBUF->SBUF DMA per exchange direction) and 2+2 circular column halos.
The 5x5x3x3 conv runs as 5 PSUM-accumulated fp16 matmuls (one per
kernel column dx) per 512-col group; the banded stationary
[3(B+4), 3(B+4)] maps out partition m = 3*(r+2)+co so PSUM/ACT output
is partition-aligned with the state tile (halo partitions get zeros, so
the elementwise tail can run on all partitions uniformly). Per block:
ACT t = relu(0.1*psum) -> DVE t2 = t + x -> DVE x' = min(t2, 1) written
back into the state tile, DVE refreshes the 4 column-halo columns, and
two Sync-queue DMAs push the boundary rows into the neighbors' halo
partitions. Block order rotates by +3 each step so next-step blocks
never wait on the tail of the previous step and the PE stays warm.
"""
import sys

sys.path.insert(0, "/opt/trn_rl_repo")

import numpy as np

N_CORES = 8
CG = 512  # matmul column group (one PSUM bank of f32)


def block_sizes(H):
    """Split H into blocks of 38 (3*(38+4)=126 <= 128 partitions)."""
    bs = []
    rem = H
    while rem > 42:
        bs.append(38)
        rem -= 38
    bs.append(rem)
    assert sum(bs) == H and all(6 <= b <= 42 for b in bs)
    return bs


def make_lhsT(W: np.ndarray, B: int) -> np.ndarray:
    """Stationary [3*(B+4), 5*3*(B+4)] fp16, one 3(B+4)-col band per dx.

    Window partition p = 3*(r+dy) + ci; out column m = 3*(r+2) + co (the
    first/last 6 columns stay zero so PSUM rows align with the state
    tile's partition layout and halo partitions receive zeros).
    """
    assert W.shape == (3, 3, 5, 5)
    S = B + 4
    KP = 3 * S
    lhsT = np.zeros((KP, 5, KP), dtype=np.float32)
    for ci in range(3):
        for r in range(B):
            for dy in range(5):
                for dx in range(5):
                    for co in range(3):
                        lhsT[3 * (r + dy) + ci, dx, 3 * (r + 2) + co] = W[
                            co, ci, dy, dx
                        ]
    return lhsT.reshape(KP, 5 * KP).astype(np.float16)


def build_body(tc, x_ap, lh_aps, y_ap, n_img, H, Wc, steps):
    """Emit the Tile program. x_ap:(n_img,3,H,Wc) f32 in; y_ap same out."""
    from contextlib import ExitStack

    from concourse import mybir

    nc = tc.nc
    f16 = mybir.dt.float16
    f32 = mybir.dt.float32
    Relu = mybir.ActivationFunctionType.Relu

    BS = block_sizes(H)
    nb = len(BS)
    OFF = np.concatenate([[0], np.cumsum(BS)])
    assert Wc % CG == 0 or Wc < CG
    n_cg = max(1, Wc // CG)
    cg = Wc // n_cg
    WF = Wc + 4

    # x/y with partition-major (row, channel) order
    xr = [x_ap[img].rearrange("c h w -> h c w") for img in range(n_img)]
    yr = [y_ap[img].rearrange("c h w -> h c w") for img in range(n_img)]

    ctx = ExitStack()
    const_pool = ctx.enter_context(tc.tile_pool(name="const", bufs=len(lh_aps)))
    state_pool = ctx.enter_context(tc.tile_pool(name="state", bufs=1))
    stage_pool = ctx.enter_context(tc.tile_pool(name="stage", bufs=6))
    act_pool = ctx.enter_context(tc.tile_pool(name="act", bufs=8))
    psum_pool = ctx.enter_context(tc.tile_pool(name="psum", bufs=8, space="PSUM"))

    lh = {}
    for Bsz, ap in lh_aps.items():
        KP = 3 * (Bsz + 4)
        t = const_pool.tile([KP, 5 * KP], f16)
        nc.sync.dma_start(t[:], ap[:, :])
        lh[Bsz] = t

    st = {}
    for img in range(n_img):
        for b in range(nb):
            KP = 3 * (BS[b] + 4)
            st[img, b] = state_pool.tile(
                [KP, WF], f16, name=f"st{img}_{b}", tag=f"st{img}_{b}"
            )

    def load_block(img, b):
        B = BS[b]
        KP = 3 * (B + 4)
        O0 = int(OFF[b])
        stg = stage_pool.tile([126, Wc], f32, tag="stage")
        if b == 0:
            nc.gpsimd.dma_start(stg[6:KP, :], xr[img][0 : B + 2, :, :])
            nc.gpsimd.dma_start(stg[0:6, :], xr[img][H - 2 : H, :, :])
        elif b == nb - 1:
            nc.gpsimd.dma_start(stg[0 : KP - 6, :], xr[img][O0 - 2 : H, :, :])
            nc.gpsimd.dma_start(stg[KP - 6 : KP, :], xr[img][0:2, :, :])
        else:
            nc.gpsimd.dma_start(stg[0:KP, :], xr[img][O0 - 2 : O0 + B + 2, :, :])
        tile = st[img, b]
        nc.vector.tensor_copy(tile[0:KP, 2 : Wc + 2], stg[0:KP, :])
        nc.vector.tensor_copy(tile[0:KP, 0:2], tile[0:KP, Wc : Wc + 2])
        nc.vector.tensor_copy(tile[0:KP, Wc + 2 : Wc + 4], tile[0:KP, 2:4])

    def emit_block(s, img, b, last):
        B = BS[b]
        KP = 3 * (B + 4)
        O0 = int(OFF[b])
        tile = st[img, b]
        lht = lh[B]
        t = act_pool.tile([126, Wc], f16, tag="t")
        for g in range(n_cg):
            psum = psum_pool.tile([126, cg], f32, tag="ps")
            for dx in range(5):
                nc.tensor.matmul(
                    psum[0:KP, :],
                    lht[:, dx * KP : (dx + 1) * KP],
                    tile[0:KP, g * cg + dx : g * cg + dx + cg],
                    start=(dx == 0),
                    stop=(dx == 4),
                )
            nc.scalar.activation(
                t[0:KP, g * cg : (g + 1) * cg], psum[0:KP, :], Relu, scale=0.1
            )
        t2 = act_pool.tile([126, Wc], f16, tag="t2")
        nc.vector.tensor_add(t2[0:KP, :], t[0:KP, :], tile[0:KP, 2 : Wc + 2])
        if last:
            stg = stage_pool.tile([126, Wc], f32, tag="stage")
            nc.vector.tensor_scalar_min(stg[0:KP, :], t2[0:KP, :], 1.0)
            q = nc.sync if b % 2 == 0 else nc.gpsimd
            q.dma_start(yr[img][O0 : O0 + B, :, :], stg[6 : 6 + 3 * B, :])
        else:
            nc.vector.tensor_scalar_min(tile[0:KP, 2 : Wc + 2], t2[0:KP, :], 1.0)
            nc.vector.tensor_copy(tile[0:KP, 0:2], tile[0:KP, Wc : Wc + 2])
            nc.vector.tensor_copy(tile[0:KP, Wc + 2 : Wc + 4], tile[0:KP, 2:4])

    def emit_pushes(img, a):
        """Exchange boundary rows between blocks a and a+1 (mod nb)."""
        b = (a + 1) % nb
        Ba = BS[a]
        ta, tb = st[img, a], st[img, b]
        # b's upper halo (rows O_b-2, O_b-1) <- a's last two rows
        nc.sync.dma_start(tb[0:6, :], ta[3 * Ba : 3 * Ba + 6, :])
        # a's lower halo (rows O_a+Ba, +Ba+1) <- b's first two rows
        nc.sync.dma_start(ta[3 * Ba + 6 : 3 * Ba + 12, :], tb[6:12, :])

    for i in range(nb):
        for img in range(n_img):
            load_block(img, i)

    for s in range(steps):
        last = s == steps - 1
        rot = (3 * s) % nb
        order = [(rot + i) % nb for i in range(nb)]
        for i, b in enumerate(order):
            for img in range(n_img):
                emit_block(s, img, b, last)
            if not last and i >= 1:
                for img in range(n_img):
                    emit_pushes(img, (b - 1) % nb)
        if not last:
            for img in range(n_img):
                emit_pushes(img, (order[0] - 1) % nb)

    ctx.close()


_PROGRAM_CACHE = {}


def _build_program(n_img, H, Wc, steps):
    key = (n_img, H, Wc, steps)
    if key in _PROGRAM_CACHE:
        return _PROGRAM_CACHE[key]
    import concourse.tile as tile
    from concourse import bacc, mybir

    nc = bacc.Bacc(
        "TRN2",
        target_bir_lowering=False,
        debug=False,
        enable_asserts=False,
        num_devices=N_CORES,
    )
    f16 = mybir.dt.float16
    f32 = mybir.dt.float32
    x_ap = nc.dram_tensor("x", (n_img, 3, H, Wc), f32, kind="ExternalInput").ap()
    lh_aps = {}
    for Bsz in sorted(set(block_sizes(H))):
        KP = 3 * (Bsz + 4)
        lh_aps[Bsz] = nc.dram_tensor(
            f"lhsT{Bsz}", (KP, 5 * KP), f16, kind="ExternalInput"
        ).ap()
    y_ap = nc.dram_tensor("y", (n_img, 3, H, Wc), f32, kind="ExternalOutput").ap()
    with tile.TileContext(nc) as tc:
        build_body(tc, x_ap, lh_aps, y_ap, n_img, H, Wc, steps)
    nc.compile()
    _PROGRAM_CACHE[key] = nc
    return nc


def _make_in_maps(x, W, n_cores=N_CORES):
    n, c, H, Wc = x.shape
    per = n // n_cores
    lhs = {Bsz: make_lhsT(W, Bsz) for Bsz in sorted(set(block_sizes(H)))}
    return [
        dict(
            {"x": x[i * per : (i + 1) * per]},
            **{f"lhsT{B}": l for B, l in lhs.items()},
        )
        for i in range(n_cores)
    ]


def kernel(x: np.ndarray, W: np.ndarray, steps) -> np.ndarray:
    from concourse.bass_utils import run_bass_kernel_spmd

    x = np.ascontiguousarray(np.asarray(x), dtype=np.float32)
    W = np.asarray(W, dtype=np.float32)
    steps = int(steps)
    n, c, H, Wc = x.shape
    assert c == 3 and n % N_CORES == 0
    per = n // N_CORES

    nc = _build_program(per, H, Wc, steps)
    in_maps = _make_in_maps(x, W)
    res = run_bass_kernel_spmd(nc, in_maps, core_ids=list(range(N_CORES)))
    out = np.concatenate([res.results[i]["y"] for i in range(N_CORES)], axis=0)
    return out.astype(np.float32)


# revision 12
# speedup vs baseline: 1.2644x; 1.0746x over previous
"""Trainium2 Bass kernel for nn_CAKernel_47459388621075.

10 steps of x = clip(x + 0.1*relu(conv5x5_circular(x, W)), 0, 1) on
x:(16,3,1024,1024) f32, W:(3,3,5,5) f32.

Sharding: batch-parallel over 8 NeuronCores (2 images/core); the circular
conv is per-image so no cross-core traffic.

Per-core strategy: the whole state lives in SBUF in fp16 for all 10 steps
(per core 2*3*1024*1024*2B = 12.6 MB), eliminating the per-step HBM
round-trip that made the v1 kernel DMA-bound and kept the PE
HAM-throttled at 1.2 GHz. H is split into 27 row blocks (26x38 + 36);
each block owns a persistent window tile [3*(B+4), 1028] in row-major
interleaved partition order p = 3*row + channel, holding B interior rows
plus 2+2 halo rows (so each halo is 6 contiguous partitions -> one
SBUF->SBUF DMA per exchange direction) and 2+2 circular column halos.
The 5x5x3x3 conv runs as 5 PSUM-accumulated fp16 matmuls (one per
kernel column dx) per 512-col group; the banded stationary
[3(B+4), 3(B+4)] maps out partition m = 3*(r+2)+co so PSUM/ACT output
is partition-aligned with the state tile (halo partitions get zeros, so
the elementwise tail can run on all partitions uniformly). Per block:
ACT t = relu(0.1*psum) -> DVE t2 = t + x -> DVE x' = min(t2, 1) written
back into the state tile, DVE refreshes the 4 column-halo columns, and
two Sync-queue DMAs push the boundary rows into the neighbors' halo
partitions. Block order rotates by +3 each step so next-step blocks
never wait on the tail of the previous step and the PE stays warm.
"""
import sys

sys.path.insert(0, "/opt/trn_rl_repo")

import numpy as np

N_CORES = 8
CG = 512  # matmul column group (one PSUM bank of f32)


def block_sizes(H):
    """Split H into blocks of 38 (3*(38+4)=126 <= 128 partitions)."""
    bs = []
    rem = H
    while rem > 42:
        bs.append(38)
        rem -= 38
    bs.append(rem)
    assert sum(bs) == H and all(6 <= b <= 42 for b in bs)
    return bs


def make_lhsT(W: np.ndarray, B: int) -> np.ndarray:
    """Stationary [3*(B+4), 5*3*(B+4)] fp16, one 3(B+4)-col band per dx.

    Window partition p = 3*(r+dy) + ci; out column m = 3*(r+2) + co (the
    first/last 6 columns stay zero so PSUM rows align with the state
    tile's partition layout and halo partitions receive zeros).
    """
    assert W.shape == (3, 3, 5, 5)
    S = B + 4
    KP = 3 * S
    lhsT = np.zeros((KP, 5, KP), dtype=np.float32)
    for ci in range(3):
        for r in range(B):
            for dy in range(5):
                for dx in range(5):
                    for co in range(3):
                        lhsT[3 * (r + dy) + ci, dx, 3 * (r + 2) + co] = W[
                            co, ci, dy, dx
                        ]
    return lhsT.reshape(KP, 5 * KP).astype(np.float16)


def build_body(tc, x_ap, lh_aps, y_ap, n_img, H, Wc, steps):
    """Emit the Tile program. x_ap:(n_img,3,H,Wc) f32 in; y_ap same out."""
    from contextlib import ExitStack

    from concourse import mybir

    nc = tc.nc
    f16 = mybir.dt.float16
    f32 = mybir.dt.float32
    Relu = mybir.ActivationFunctionType.Relu

    BS = block_sizes(H)
    nb = len(BS)
    OFF = np.concatenate([[0], np.cumsum(BS)])
    assert Wc % CG == 0 or Wc < CG
    n_cg = max(1, Wc // CG)
    cg = Wc // n_cg
    WF = Wc + 4

    # x/y with partition-major (row, channel) order
    xr = [x_ap[img].rearrange("c h w -> h c w") for img in range(n_img)]
    yr = [y_ap[img].rearrange("c h w -> h c w") for img in range(n_img)]

    ctx = ExitStack()
    const_pool = ctx.enter_context(tc.tile_pool(name="const", bufs=len(lh_aps)))
    state_pool = ctx.enter_context(tc.tile_pool(name="state", bufs=1))
    stage_pool = ctx.enter_context(tc.tile_pool(name="stage", bufs=6))
    act_pool = ctx.enter_context(tc.tile_pool(name="act", bufs=8))
    psum_pool = ctx.enter_context(tc.tile_pool(name="psum", bufs=8, space="PSUM"))

    lh = {}
    for Bsz, ap in lh_aps.items():
        KP = 3 * (Bsz + 4)
        t = const_pool.tile([KP, 5 * KP], f16)
        nc.sync.dma_start(t[:], ap[:, :])
        lh[Bsz] = t

    st = {}
    for img in range(n_img):
        for b in range(nb):
            KP = 3 * (BS[b] + 4)
            st[img, b] = state_pool.tile(
                [KP, WF], f16, name=f"st{img}_{b}", tag=f"st{img}_{b}"
            )

    def load_block(img, b):
        B = BS[b]
        KP = 3 * (B + 4)
        O0 = int(OFF[b])
        stg = stage_pool.tile([126, Wc], f32, tag="stage")
        if b == 0:
            nc.gpsimd.dma_start(stg[6:KP, :], xr[img][0 : B + 2, :, :])
            nc.gpsimd.dma_start(stg[0:6, :], xr[img][H - 2 : H, :, :])
        elif b == nb - 1:
            nc.gpsimd.dma_start(stg[0 : KP - 6, :], xr[img][O0 - 2 : H, :, :])
            nc.gpsimd.dma_start(stg[KP - 6 : KP, :], xr[img][0:2, :, :])
        else:
            nc.gpsimd.dma_start(stg[0:KP, :], xr[img][O0 - 2 : O0 + B + 2, :, :])
        tile = st[img, b]
        nc.vector.tensor_copy(tile[0:KP, 2 : Wc + 2], stg[0:KP, :])
        nc.vector.tensor_copy(tile[0:KP, 0:2], tile[0:KP, Wc : Wc + 2])
        nc.vector.tensor_copy(tile[0:KP, Wc + 2 : Wc + 4], tile[0:KP, 2:4])

    def emit_block(s, img, b, last):
        B = BS[b]
        KP = 3 * (B + 4)
        O0 = int(OFF[b])
        tile = st[img, b]
        lht = lh[B]
        t = act_pool.tile([126, Wc], f16, tag="t")
        for g in range(n_cg):
            psum = psum_pool.tile([126, cg], f32, tag="ps")
            for dx in range(5):
                nc.tensor.matmul(
                    psum[0:KP, :],
                    lht[:, dx * KP : (dx + 1) * KP],
                    tile[0:KP, g * cg + dx : g * cg + dx + cg],
                    start=(dx == 0),
                    stop=(dx == 4),
                )
            nc.scalar.activation(
                t[0:KP, g * cg : (g + 1) * cg], psum[0:KP, :], Relu, scale=0.1
            )
        t2 = act_pool.tile([126, Wc], f16, tag="t2")
        nc.vector.tensor_add(t2[0:KP, :], t[0:KP, :], tile[0:KP, 2 : Wc + 2])
        if last:
            stg = stage_pool.tile([126, Wc], f32, tag="stage")
            nc.vector.tensor_scalar_min(stg[0:KP, :], t2[0:KP, :], 1.0)
            nc.gpsimd.dma_start(yr[img][O0 : O0 + B, :, :], stg[6 : 6 + 3 * B, :])
        else:
            nc.vector.tensor_scalar_min(tile[0:KP, 2 : Wc + 2], t2[0:KP, :], 1.0)
            nc.vector.tensor_copy(tile[0:KP, 0:2], tile[0:KP, Wc : Wc + 2])
            nc.vector.tensor_copy(tile[0:KP, Wc + 2 : Wc + 4], tile[0:KP, 2:4])

    def emit_pushes(img, a):
        """Exchange boundary rows between blocks a and a+1 (mod nb)."""
        b = (a + 1) % nb
        Ba = BS[a]
        ta, tb = st[img, a], st[img, b]
        # b's upper halo (rows O_b-2, O_b-1) <- a's last two rows
        nc.sync.dma_start(tb[0:6, :], ta[3 * Ba : 3 * Ba + 6, :])
        # a's lower halo (rows O_a+Ba, +Ba+1) <- b's first two rows
        nc.sync.dma_start(ta[3 * Ba + 6 : 3 * Ba + 12, :], tb[6:12, :])

    for i in range(nb):
        for img in range(n_img):
            load_block(img, i)

    for s in range(steps):
        last = s == steps - 1
        rot = (3 * s) % nb
        order = [(rot + i) % nb for i in range(nb)]
        for i, b in enumerate(order):
            for img in range(n_img):
                emit_block(s, img, b, last)
            if not last and i >= 1:
                for img in range(n_img):
                    emit_pushes(img, (b - 1) % nb)
        if not last:
            for img in range(n_img):
                emit_pushes(img, (order[0] - 1) % nb)

    ctx.close()


_PROGRAM_CACHE = {}


def _build_program(n_img, H, Wc, steps):
    key = (n_img, H, Wc, steps)
    if key in _PROGRAM_CACHE:
        return _PROGRAM_CACHE[key]
    import concourse.tile as tile
    from concourse import bacc, mybir

    nc = bacc.Bacc(
        "TRN2",
        target_bir_lowering=False,
        debug=False,
        enable_asserts=False,
        num_devices=N_CORES,
    )
    f16 = mybir.dt.float16
    f32 = mybir.dt.float32
    x_ap = nc.dram_tensor("x", (n_img, 3, H, Wc), f32, kind="ExternalInput").ap()
    lh_aps = {}
    for Bsz in sorted(set(block_sizes(H))):
        KP = 3 * (Bsz + 4)
        lh_aps[Bsz] = nc.dram_tensor(
            f"lhsT{Bsz}", (KP, 5 * KP), f16, kind="ExternalInput"
        ).ap()
    y_ap = nc.dram_tensor("y", (n_img, 3, H, Wc), f32, kind="ExternalOutput").ap()
    with tile.TileContext(nc) as tc:
        build_body(tc, x_ap, lh_aps, y_ap, n_img, H, Wc, steps)
    nc.compile()
    _PROGRAM_CACHE[key] = nc
    return nc


def _make_in_maps(x, W, n_cores=N_CORES):
    n, c, H, Wc = x.shape
    per = n // n_cores
    lhs = {Bsz: make_lhsT(W, Bsz) for Bsz in sorted(set(block_sizes(H)))}
    return [
        dict(
            {"x": x[i * per : (i + 1) * per]},
            **{f"lhsT{B}": l for B, l in lhs.items()},
        )
        for i in range(n_cores)
    ]


def kernel(x: np.ndarray, W: np.ndarray, steps) -> np.ndarray:
    from concourse.bass_utils import run_bass_kernel_spmd

    x = np.ascontiguousarray(np.asarray(x), dtype=np.float32)
    W = np.asarray(W, dtype=np.float32)
    steps = int(steps)
    n, c, H, Wc = x.shape
    assert c == 3 and n % N_CORES == 0
    per = n // N_CORES

    nc = _build_program(per, H, Wc, steps)
    in_maps = _make_in_maps(x, W)
    res = run_bass_kernel_spmd(nc, in_maps, core_ids=list(range(N_CORES)))
    out = np.concatenate([res.results[i]["y"] for i in range(N_CORES)], axis=0)
    return out.astype(np.float32)


# revision 15
# speedup vs baseline: 1.3113x; 1.0371x over previous
"""Trainium2 Bass kernel for nn_CAKernel_47459388621075.

10 steps of x = clip(x + 0.1*relu(conv5x5_circular(x, W)), 0, 1) on
x:(16,3,1024,1024) f32, W:(3,3,5,5) f32.

Sharding: batch-parallel over 8 NeuronCores (2 images/core); the circular
conv is per-image so no cross-core traffic.

Per-core strategy: the whole state lives in SBUF in fp16 for all 10 steps
(per core 2*3*1024*1024*2B = 12.6 MB), eliminating the per-step HBM
round-trip that made the v1 kernel DMA-bound and kept the PE
HAM-throttled at 1.2 GHz. H is split into 27 row blocks (26x38 + 36);
each block owns a persistent window tile [3*(B+4), 1028] in row-major
interleaved partition order p = 3*row + channel, holding B interior rows
plus 2+2 halo rows (so each halo is 6 contiguous partitions -> one
SBUF->SBUF DMA per exchange direction) and 2+2 circular column halos.
The 5x5x3x3 conv runs as 5 PSUM-accumulated fp16 matmuls (one per
kernel column dx) per 512-col group; the banded stationary
[3(B+4), 3(B+4)] maps out partition m = 3*(r+2)+co so PSUM/ACT output
is partition-aligned with the state tile (halo partitions get zeros, so
the elementwise tail can run on all partitions uniformly). Per block:
ACT t = relu(0.1*psum) -> DVE t2 = t + x -> DVE x' = min(t2, 1) written
back into the state tile, DVE refreshes the 4 column-halo columns, and
two Sync-queue DMAs push the boundary rows into the neighbors' halo
partitions. Block order rotates by +3 each step so next-step blocks
never wait on the tail of the previous step and the PE stays warm.
"""
import sys

sys.path.insert(0, "/opt/trn_rl_repo")

import numpy as np

N_CORES = 8
CG = 512  # matmul column group (one PSUM bank of f32)


def block_sizes(H):
    """Split H into blocks of 38 (3*(38+4)=126 <= 128 partitions)."""
    bs = []
    rem = H
    while rem > 42:
        bs.append(38)
        rem -= 38
    bs.append(rem)
    assert sum(bs) == H and all(6 <= b <= 42 for b in bs)
    return bs


def make_lhsT(W: np.ndarray, B: int) -> np.ndarray:
    """Stationary [3*(B+4), 5*3*(B+4)] fp16, one 3(B+4)-col band per dx.

    Window partition p = 3*(r+dy) + ci; out column m = 3*(r+2) + co (the
    first/last 6 columns stay zero so PSUM rows align with the state
    tile's partition layout and halo partitions receive zeros).
    """
    assert W.shape == (3, 3, 5, 5)
    S = B + 4
    KP = 3 * S
    lhsT = np.zeros((KP, 5, KP), dtype=np.float32)
    for ci in range(3):
        for r in range(B):
            for dy in range(5):
                for dx in range(5):
                    for co in range(3):
                        lhsT[3 * (r + dy) + ci, dx, 3 * (r + 2) + co] = W[
                            co, ci, dy, dx
                        ]
    return lhsT.reshape(KP, 5 * KP).astype(np.float16)


def build_body(tc, x_ap, lh_aps, y_ap, n_img, H, Wc, steps):
    """Emit the Tile program. x_ap:(n_img,3,H,Wc) f32 in; y_ap same out."""
    from contextlib import ExitStack

    from concourse import mybir

    nc = tc.nc
    f16 = mybir.dt.float16
    f32 = mybir.dt.float32
    Relu = mybir.ActivationFunctionType.Relu

    BS = block_sizes(H)
    nb = len(BS)
    OFF = np.concatenate([[0], np.cumsum(BS)])
    assert Wc % CG == 0 or Wc < CG
    n_cg = max(1, Wc // CG)
    cg = Wc // n_cg
    WF = Wc + 4

    # x/y with partition-major (row, channel) order
    xr = [x_ap[img].rearrange("c h w -> h c w") for img in range(n_img)]
    yr = [y_ap[img].rearrange("c h w -> h c w") for img in range(n_img)]

    ctx = ExitStack()
    const_pool = ctx.enter_context(tc.tile_pool(name="const", bufs=len(lh_aps)))
    state_pool = ctx.enter_context(tc.tile_pool(name="state", bufs=1))
    stage_pool = ctx.enter_context(tc.tile_pool(name="stage", bufs=6))
    act_pool = ctx.enter_context(tc.tile_pool(name="act", bufs=8))
    psum_pool = ctx.enter_context(tc.tile_pool(name="psum", bufs=8, space="PSUM"))

    lh = {}
    for Bsz, ap in lh_aps.items():
        KP = 3 * (Bsz + 4)
        t = const_pool.tile([KP, 5 * KP], f16)
        nc.sync.dma_start(t[:], ap[:, :])
        lh[Bsz] = t

    st = {}
    for img in range(n_img):
        for b in range(nb):
            KP = 3 * (BS[b] + 4)
            st[img, b] = state_pool.tile(
                [KP, WF], f16, name=f"st{img}_{b}", tag=f"st{img}_{b}"
            )

    def load_block(img, b):
        B = BS[b]
        KP = 3 * (B + 4)
        O0 = int(OFF[b])
        stg = stage_pool.tile([126, Wc], f32, tag="stage")
        if b == 0:
            nc.gpsimd.dma_start(stg[6:KP, :], xr[img][0 : B + 2, :, :])
            nc.gpsimd.dma_start(stg[0:6, :], xr[img][H - 2 : H, :, :])
        elif b == nb - 1:
            nc.gpsimd.dma_start(stg[0 : KP - 6, :], xr[img][O0 - 2 : H, :, :])
            nc.gpsimd.dma_start(stg[KP - 6 : KP, :], xr[img][0:2, :, :])
        else:
            nc.gpsimd.dma_start(stg[0:KP, :], xr[img][O0 - 2 : O0 + B + 2, :, :])
        tile = st[img, b]
        nc.vector.tensor_copy(tile[0:KP, 2 : Wc + 2], stg[0:KP, :])
        nc.scalar.copy(tile[0:KP, 0:2], tile[0:KP, Wc : Wc + 2])
        nc.scalar.copy(tile[0:KP, Wc + 2 : Wc + 4], tile[0:KP, 2:4])

    def emit_block(s, img, b, last):
        B = BS[b]
        KP = 3 * (B + 4)
        O0 = int(OFF[b])
        tile = st[img, b]
        lht = lh[B]
        t = act_pool.tile([126, Wc], f16, tag="t")
        for g in range(n_cg):
            psum = psum_pool.tile([126, cg], f32, tag="ps")
            for dx in range(5):
                nc.tensor.matmul(
                    psum[0:KP, :],
                    lht[:, dx * KP : (dx + 1) * KP],
                    tile[0:KP, g * cg + dx : g * cg + dx + cg],
                    start=(dx == 0),
                    stop=(dx == 4),
                )
            nc.scalar.activation(
                t[0:KP, g * cg : (g + 1) * cg], psum[0:KP, :], Relu, scale=0.1
            )
        t2 = act_pool.tile([126, Wc], f16, tag="t2")
        nc.vector.tensor_add(t2[0:KP, :], t[0:KP, :], tile[0:KP, 2 : Wc + 2])
        nc.vector.tensor_scalar_min(tile[0:KP, 2 : Wc + 2], t2[0:KP, :], 1.0)
        if last:
            nc.gpsimd.dma_start(
                yr[img][O0 : O0 + B, :, :], tile[6 : 6 + 3 * B, 2 : Wc + 2]
            )
        else:
            nc.vector.tensor_copy(tile[0:KP, 0:2], tile[0:KP, Wc : Wc + 2])
            nc.vector.tensor_copy(tile[0:KP, Wc + 2 : Wc + 4], tile[0:KP, 2:4])

    def emit_pushes(img, a):
        """Exchange boundary rows between blocks a and a+1 (mod nb)."""
        b = (a + 1) % nb
        Ba = BS[a]
        ta, tb = st[img, a], st[img, b]
        # b's upper halo (rows O_b-2, O_b-1) <- a's last two rows
        nc.sync.dma_start(tb[0:6, :], ta[3 * Ba : 3 * Ba + 6, :])
        # a's lower halo (rows O_a+Ba, +Ba+1) <- b's first two rows
        nc.sync.dma_start(ta[3 * Ba + 6 : 3 * Ba + 12, :], tb[6:12, :])

    for i in range(nb):
        for img in range(n_img):
            load_block(img, i)

    for s in range(steps):
        last = s == steps - 1
        rot = (3 * s) % nb
        order = [(rot + i) % nb for i in range(nb)]
        for i, b in enumerate(order):
            for img in range(n_img):
                emit_block(s, img, b, last)
            if not last and i >= 1:
                for img in range(n_img):
                    emit_pushes(img, (b - 1) % nb)
        if not last:
            for img in range(n_img):
                emit_pushes(img, (order[0] - 1) % nb)

    ctx.close()


_PROGRAM_CACHE = {}


def _build_program(n_img, H, Wc, steps):
    key = (n_img, H, Wc, steps)
    if key in _PROGRAM_CACHE:
        return _PROGRAM_CACHE[key]
    import concourse.tile as tile
    from concourse import bacc, mybir

    nc = bacc.Bacc(
        "TRN2",
        target_bir_lowering=False,
        debug=False,
        enable_asserts=False,
        num_devices=N_CORES,
    )
    f16 = mybir.dt.float16
    f32 = mybir.dt.float32
    x_ap = nc.dram_tensor("x", (n_img, 3, H, Wc), f32, kind="ExternalInput").ap()
    lh_aps = {}
    for Bsz in sorted(set(block_sizes(H))):
        KP = 3 * (Bsz + 4)
        lh_aps[Bsz] = nc.dram_tensor(
            f"lhsT{Bsz}", (KP, 5 * KP), f16, kind="ExternalInput"
        ).ap()
    y_ap = nc.dram_tensor("y", (n_img, 3, H, Wc), f16, kind="ExternalOutput").ap()
    with tile.TileContext(nc) as tc:
        build_body(tc, x_ap, lh_aps, y_ap, n_img, H, Wc, steps)
    nc.compile()
    _PROGRAM_CACHE[key] = nc
    return nc


def _make_in_maps(x, W, n_cores=N_CORES):
    n, c, H, Wc = x.shape
    per = n // n_cores
    lhs = {Bsz: make_lhsT(W, Bsz) for Bsz in sorted(set(block_sizes(H)))}
    return [
        dict(
            {"x": x[i * per : (i + 1) * per]},
            **{f"lhsT{B}": l for B, l in lhs.items()},
        )
        for i in range(n_cores)
    ]


def kernel(x: np.ndarray, W: np.ndarray, steps) -> np.ndarray:
    from concourse.bass_utils import run_bass_kernel_spmd

    x = np.ascontiguousarray(np.asarray(x), dtype=np.float32)
    W = np.asarray(W, dtype=np.float32)
    steps = int(steps)
    n, c, H, Wc = x.shape
    assert c == 3 and n % N_CORES == 0
    per = n // N_CORES

    nc = _build_program(per, H, Wc, steps)
    in_maps = _make_in_maps(x, W)
    res = run_bass_kernel_spmd(nc, in_maps, core_ids=list(range(N_CORES)))
    out = np.concatenate([res.results[i]["y"] for i in range(N_CORES)], axis=0)
    return out.astype(np.float32)
